# revision 7
# baseline (speedup 1.0000x reference)
"""Distributed GQA attention kernel for 8 TRN2 NeuronCores.

Problem: B=1, S=2048, D=4096, H=32 q-heads, KV=8 kv-heads, HD=128.
  q = rope(x@wq.T), k = rope(x@wk.T), v = x@wv.T
  out = softmax(causal(q@k.T/sqrt(HD))) @ v @ wo.T

Sharding: tensor-parallel over heads. Core c owns q-heads 4c..4c+3 and
kv-head c. Device-side per core:
  phase 1: QT/KT (rope'd, [hd, s] layout) + VT projections; V tiles
           ([t, hd]) via DMA transpose
  phase 2: causal attention producing attnT chunks; per s-chunk
           AllGather (overlapped) + out-proj slice = wot.T @ attnT
Host side: layout prep (transposes, bf16 cast, rope tables) + final
concat/transpose of the 8 out.T slices.
"""

import math
import numpy as np
import ml_dtypes

BF = ml_dtypes.bfloat16

B, S, D = 1, 2048, 4096
H, KV, HD = 32, 8, 128
NCORES = 8
HL = H // NCORES            # 4 local q heads
QW = HL * HD                # 512 local q width
SC = 512                    # s-chunk width
NSC = S // SC               # 4 s-chunks
KD = 32                     # d-dim k-tiles (4096/128)
NT = S // 128               # 16 t-tiles
SCALE = 1.0 / math.sqrt(HD)
NEG = -30000.0


def _build_nc():
    import concourse.bass as bass
    import concourse.mybir as mybir
    from concourse import bacc, tile

    dt = mybir.dt
    nc = bacc.Bacc()

    xt_d = nc.declare_dram_parameter("xt", [D, S], dt.bfloat16, isOutput=False)
    wqt_d = nc.declare_dram_parameter("wqt", [D, QW], dt.bfloat16, isOutput=False)
    wkt_d = nc.declare_dram_parameter("wkt", [D, HD], dt.bfloat16, isOutput=False)
    wvt_d = nc.declare_dram_parameter("wvt", [D, HD], dt.bfloat16, isOutput=False)
    wot_d = nc.declare_dram_parameter("wot", [D, QW], dt.bfloat16, isOutput=False)
    cosd_d = nc.declare_dram_parameter("cosd", [HD, S], dt.bfloat16, isOutput=False)
    sind_d = nc.declare_dram_parameter("sind", [HD, S], dt.bfloat16, isOutput=False)
    swapt_d = nc.declare_dram_parameter("swapt", [HD, HD], dt.bfloat16, isOutput=False)
    ident_d = nc.declare_dram_parameter("ident", [HD, HD], dt.bfloat16, isOutput=False)
    dmask_d = nc.declare_dram_parameter("dmask", [128, 128], dt.float32, isOutput=False)
    onesc_d = nc.declare_dram_parameter("onesc", [128, 1], dt.bfloat16, isOutput=False)
    onesr_d = nc.declare_dram_parameter("onesr", [1, 128], dt.float32, isOutput=False)
    out_d = nc.declare_dram_parameter("out_t", [QW, S], dt.float32, isOutput=True)

    with tile.TileContext(nc) as tc:
        with (
            tc.tile_pool(name="const", bufs=1) as cpool,
            tc.tile_pool(name="qkv", bufs=1) as qkvpool,
            tc.tile_pool(name="att", bufs=1) as attpool,
            tc.tile_pool(name="dram", bufs=1, space="DRAM") as dpool,
        ):
            # ---- resident constants / weights ----
            wqt = cpool.tile([128, KD, QW], dt.bfloat16)
            wkt = cpool.tile([128, KD, HD], dt.bfloat16)
            wvt = cpool.tile([128, KD, HD], dt.bfloat16)
            cosd = cpool.tile([HD, S], dt.bfloat16)
            sind = cpool.tile([HD, S], dt.bfloat16)
            swapt = cpool.tile([HD, HD], dt.bfloat16)
            ident = cpool.tile([HD, HD], dt.bfloat16)
            dmask = cpool.tile([128, 128], dt.float32)
            onesc = cpool.tile([128, 1], dt.bfloat16)
            onesr = cpool.tile([1, 128], dt.float32)

            wqt_r = wqt_d[:, :].rearrange("(k p) n -> p k n", p=128)
            # split weight loads so the first matmuls can start early
            for kg in range(4):
                ksl = slice(kg * 8, (kg + 1) * 8)
                nc.sync.dma_start(wqt[:, ksl, :], wqt_r[:, ksl, :])
            nc.sync.dma_start(wkt[:], wkt_d[:, :].rearrange("(k p) n -> p k n", p=128))
            nc.sync.dma_start(wvt[:], wvt_d[:, :].rearrange("(k p) n -> p k n", p=128))
            nc.sync.dma_start(cosd[:], cosd_d[:, :])
            nc.sync.dma_start(sind[:], sind_d[:, :])
            nc.sync.dma_start(swapt[:], swapt_d[:, :])
            nc.sync.dma_start(ident[:], ident_d[:, :])
            nc.sync.dma_start(dmask[:], dmask_d[:, :])
            nc.sync.dma_start(onesc[:], onesc_d[:, :])
            nc.sync.dma_start(onesr[:], onesr_d[:, :])

            # ---- persistent activations ----
            qt = [qkvpool.tile([HD, S], dt.bfloat16, name=f"qt{h}", tag=f"qt{h}")
                  for h in range(HL)]
            kt = qkvpool.tile([HD, S], dt.bfloat16)
            vt = qkvpool.tile([HD, S], dt.bfloat16)
            vv = qkvpool.tile([128, NT, HD], dt.bfloat16)   # [t_part, ti, hd]
            att = [attpool.tile([HD, S], dt.bfloat16, name=f"att{h}", tag=f"att{h}")
                   for h in range(HL)]

            xt_r = xt_d[:, :].rearrange("(k p) s -> p k s", p=128)

            # ================= phase 1: projections + rope =================
            with (
                tc.tile_pool(name="xc", bufs=2) as xpool,
                tc.tile_pool(name="p1", bufs=3, space="PSUM") as pp1,
                tc.tile_pool(name="pr", bufs=2, space="PSUM") as ppr,
                tc.tile_pool(name="rtmp", bufs=3) as rtpool,
            ):
                for sc in range(NSC):
                    ssl = slice(sc * SC, (sc + 1) * SC)
                    xc = xpool.tile([128, KD, SC], dt.bfloat16)
                    for kg in range(4):
                        ksl = slice(kg * 8, (kg + 1) * 8)
                        nc.sync.dma_start(xc[:, ksl, :], xt_r[:, ksl, ssl])

                    # 4 Q heads (rope), K (rope), V (plain) — all [hd, s]
                    for hi in range(HL + 2):
                        ps = pp1.tile([128, SC], dt.float32)
                        for k in range(KD):
                            if hi < HL:
                                lhs = wqt[:, k, hi * HD:(hi + 1) * HD]
                            elif hi == HL:
                                lhs = wkt[:, k, :]
                            else:
                                lhs = wvt[:, k, :]
                            nc.tensor.matmul(ps[:], lhs, xc[:, k, :],
                                             start=(k == 0), stop=(k == KD - 1))
                        if hi == HL + 1:
                            nc.scalar.copy(vt[:, ssl], ps[:])
                            continue
                        # rope: out = q*cos + rot(q)*sin, rot via swap-matmul
                        qs = rtpool.tile([128, SC], dt.bfloat16, tag="ropeqs")
                        qc = rtpool.tile([128, SC], dt.bfloat16, tag="ropeqc")
                        nc.vector.tensor_mul(qs[:], ps[:], sind[:, ssl])
                        nc.vector.tensor_mul(qc[:], ps[:], cosd[:, ssl])
                        ps2 = ppr.tile([128, SC], dt.float32)
                        nc.tensor.matmul(ps2[:], swapt[:], qs[:], start=True, stop=False)
                        nc.tensor.matmul(ps2[:], ident[:], qc[:], start=False, stop=True)
                        dst = qt[hi] if hi < HL else kt
                        nc.scalar.copy(dst[:, ssl], ps2[:])

                    # V tiles in [t, hd] layout via DMA transpose
                    for vtile in range(4):
                        ti = sc * 4 + vtile
                        nc.sync.dma_start_transpose(
                            vv[:, ti, :], vt[:, ti * 128:(ti + 1) * 128])

            # ============ phase 2+3: attention, allgather, out-proj ============
            with (
                tc.tile_pool(name="wo", bufs=1) as wopool,
                tc.tile_pool(name="agc", bufs=2) as agpool,
                tc.tile_pool(name="st", bufs=2, space="PSUM") as stpool,
                tc.tile_pool(name="pv", bufs=2, space="PSUM") as pvpool,
                tc.tile_pool(name="rs", bufs=1, space="PSUM") as rspool,
                tc.tile_pool(name="bc", bufs=1, space="PSUM") as bcpool,
                tc.tile_pool(name="p3", bufs=2, space="PSUM") as pp3,
                tc.tile_pool(name="pt", bufs=4) as ptpool,
                tc.tile_pool(name="ep", bufs=2) as eppool,
                tc.tile_pool(name="o3", bufs=3) as opool,
            ):
                wot = wopool.tile([128, KD, QW], dt.bfloat16)
                nc.sync.dma_start(wot[:], wot_d[:, :].rearrange("(k p) n -> p k n", p=128))
                ag_r = None
                for sc in range(NSC):
                    ssl = slice(sc * SC, (sc + 1) * SC)
                    n_t = sc * 4 + 4
                    for h in range(HL):
                        pv = pvpool.tile([128, SC], dt.float32)
                        rs = rspool.tile([1, SC], dt.float32)
                        for ti in range(n_t):
                            st = stpool.tile([128, SC], dt.float32)
                            nc.tensor.matmul(st[:], kt[:, ti * 128:(ti + 1) * 128],
                                             qt[h][:, ssl], start=True, stop=True)
                            d_off = ti * 128 - sc * SC
                            if d_off >= 0:
                                nc.vector.tensor_add(st[:, d_off:d_off + 128],
                                                     st[:, d_off:d_off + 128],
                                                     dmask[:])
                            pt = ptpool.tile([128, SC], dt.bfloat16)
                            nc.scalar.activation(pt[:], st[:],
                                                 mybir.ActivationFunctionType.Exp,
                                                 scale=SCALE)
                            if d_off > 0:
                                nc.gpsimd.memset(pt[:, :d_off], 0.0)
                            nc.tensor.matmul(rs[:], onesc[:], pt[:],
                                             start=(ti == 0), stop=(ti == n_t - 1))
                            nc.tensor.matmul(pv[:], vv[:, ti, :], pt[:],
                                             start=(ti == 0), stop=(ti == n_t - 1))
                        # epilogue: normalize columns by 1/rowsum
                        rec = eppool.tile([1, SC], dt.float32, tag="rec")
                        nc.vector.reciprocal(rec[:], rs[:])
                        bc = bcpool.tile([128, SC], dt.float32)
                        nc.tensor.matmul(bc[:], onesr[:], rec[:], start=True, stop=True)
                        bcs = eppool.tile([128, SC], dt.float32, tag="bcs")
                        nc.scalar.copy(bcs[:], bc[:])
                        nc.vector.tensor_mul(att[h][:, ssl], pv[:], bcs[:])

                    # chunked AllGather of this s-chunk's attnT
                    ag_in = dpool.tile([QW, SC], dt.bfloat16,
                                       name=f"agi{sc}", tag=f"agi{sc}")
                    ag_out = dpool.tile([NCORES * QW, SC], dt.bfloat16,
                                        name=f"ago{sc}", tag=f"ago{sc}",
                                        addr_space="Shared")
                    for h in range(HL):
                        nc.sync.dma_start(ag_in[h * HD:(h + 1) * HD, :],
                                          att[h][:, ssl])
                    nc.gpsimd.collective_compute(
                        "AllGather",
                        mybir.AluOpType.bypass,
                        replica_groups=[list(range(NCORES))],
                        ins=[ag_in.opt()],
                        outs=[ag_out.opt()],
                    )
                    ag_r = ag_out[:, :].rearrange("(k p) s -> p k s", p=128)

                    # out-proj for this s-chunk
                    agcs = []
                    for kc in range(2):
                        agc = agpool.tile([128, KD // 2, SC], dt.bfloat16)
                        nc.sync.dma_start(agc[:], ag_r[:, kc * 16:(kc + 1) * 16, :])
                        agcs.append(agc)
                    for oc in range(4):
                        ps = pp3.tile([128, SC], dt.float32)
                        for k in range(KD):
                            nc.tensor.matmul(ps[:], wot[:, k, oc * 128:(oc + 1) * 128],
                                             agcs[k // 16][:, k % 16, :],
                                             start=(k == 0), stop=(k == KD - 1))
                        ot = opool.tile([128, SC], dt.float32)
                        nc.vector.tensor_copy(ot[:], ps[:])
                        nc.sync.dma_start(out_d[oc * 128:(oc + 1) * 128, ssl], ot[:])
    if not nc.is_finalized():
        nc.finalize()
    return nc


_CACHE = {}


def _get_nc():
    if "nc" not in _CACHE:
        _CACHE["nc"] = _build_nc()
    return _CACHE["nc"]


def _prep_in_maps(x, wq, wk, wv, wo, freqs_cos, freqs_sin):
    xt = np.ascontiguousarray(x.reshape(S, D).T).astype(BF)
    cosd = np.repeat(np.asarray(freqs_cos, np.float32).T, 2, axis=0).astype(BF)
    sind = np.repeat(np.asarray(freqs_sin, np.float32).T, 2, axis=0).astype(BF)
    swapt = np.zeros((HD, HD), np.float32)
    for i in range(HD // 2):
        swapt[2 * i + 1, 2 * i] = -1.0
        swapt[2 * i, 2 * i + 1] = 1.0
    swapt = swapt.astype(BF)
    ident = np.eye(HD, dtype=np.float32).astype(BF)
    t_idx = np.arange(128)[:, None]
    s_idx = np.arange(128)[None, :]
    dmask = np.where(s_idx >= t_idx, 0.0, NEG).astype(np.float32)
    onesc = np.ones((128, 1), np.float32).astype(BF)
    onesr = np.ones((1, 128), np.float32)

    wq = np.asarray(wq, np.float32)
    wk = np.asarray(wk, np.float32)
    wv = np.asarray(wv, np.float32)
    wo = np.asarray(wo, np.float32)

    in_maps = []
    for c in range(NCORES):
        qsl = slice(QW * c, QW * (c + 1))
        ksl = slice(HD * c, HD * (c + 1))
        in_maps.append({
            "xt": xt,
            "wqt": np.ascontiguousarray(wq[qsl].T).astype(BF),
            "wkt": np.ascontiguousarray(wk[ksl].T).astype(BF),
            "wvt": np.ascontiguousarray(wv[ksl].T).astype(BF),
            "wot": np.ascontiguousarray(wo[qsl].T).astype(BF),
            "cosd": cosd, "sind": sind, "swapt": swapt, "ident": ident,
            "dmask": dmask, "onesc": onesc, "onesr": onesr,
        })
    return in_maps


def run(inputs, trace=False):
    from concourse.bass_utils import run_bass_kernel_spmd
    nc = _get_nc()
    in_maps = _prep_in_maps(
        inputs["x"], inputs["wq"], inputs["wk"], inputs["wv"], inputs["wo"],
        inputs["freqs_cos"], inputs["freqs_sin"])
    res = run_bass_kernel_spmd(nc, in_maps, core_ids=list(range(NCORES)),
                               trace=trace)
    shards = [np.asarray(res.results[c]["out_t"], np.float32)
              for c in range(NCORES)]
    full = np.concatenate(shards, axis=0)          # [4096, 2048]
    out = np.ascontiguousarray(full.T)[None]       # [1, 2048, 4096]
    return out.astype(np.float32), res


def kernel(**inputs):
    out, _ = run(inputs, trace=False)
    return out


# revision 11
# speedup vs baseline: 1.1633x; 1.1633x over previous
"""Distributed GQA attention kernel for 8 TRN2 NeuronCores.

Problem: B=1, S=2048, D=4096, H=32 q-heads, KV=8 kv-heads, HD=128.
  q = rope(x@wq.T), k = rope(x@wk.T), v = x@wv.T
  out = softmax(causal(q@k.T/sqrt(HD))) @ v @ wo.T

Sharding: tensor-parallel over heads. Core c owns q-heads 4c..4c+3 and
kv-head c. Device-side per core:
  phase 1: QT/KT (rope'd, [hd, s] layout) + VT projections; V tiles
           ([t, hd]) via DMA transpose
  phase 2: causal attention producing attnT chunks; per s-chunk
           AllGather (overlapped) + out-proj slice = wot.T @ attnT
Host side: layout prep (transposes, bf16 cast, rope tables) + final
concat/transpose of the 8 out.T slices.
"""

import math
import numpy as np
import ml_dtypes

BF = ml_dtypes.bfloat16

B, S, D = 1, 2048, 4096
H, KV, HD = 32, 8, 128
NCORES = 8
HL = H // NCORES            # 4 local q heads
QW = HL * HD                # 512 local q width
SC = 512                    # s-chunk width
NSC = S // SC               # 4 s-chunks
KD = 32                     # d-dim k-tiles (4096/128)
NT = S // 128               # 16 t-tiles
SCALE = 1.0 / math.sqrt(HD)
NEG = -30000.0


def _build_nc():
    import concourse.bass as bass
    import concourse.mybir as mybir
    from concourse import bacc, tile

    dt = mybir.dt
    nc = bacc.Bacc()

    xt_d = nc.declare_dram_parameter("xt", [D, S], dt.bfloat16, isOutput=False)
    wqt_d = nc.declare_dram_parameter("wqt", [D, QW], dt.bfloat16, isOutput=False)
    wkt_d = nc.declare_dram_parameter("wkt", [D, HD], dt.bfloat16, isOutput=False)
    wvt_d = nc.declare_dram_parameter("wvt", [D, HD], dt.bfloat16, isOutput=False)
    wot_d = nc.declare_dram_parameter("wot", [D, QW], dt.bfloat16, isOutput=False)
    cosd_d = nc.declare_dram_parameter("cosd", [HD, S], dt.bfloat16, isOutput=False)
    sind_d = nc.declare_dram_parameter("sind", [HD, S], dt.bfloat16, isOutput=False)
    swapt_d = nc.declare_dram_parameter("swapt", [HD, HD], dt.bfloat16, isOutput=False)
    ident_d = nc.declare_dram_parameter("ident", [HD, HD], dt.bfloat16, isOutput=False)
    dmask_d = nc.declare_dram_parameter("dmask", [128, 128], dt.float32, isOutput=False)
    onesc_d = nc.declare_dram_parameter("onesc", [128, 1], dt.bfloat16, isOutput=False)
    onesr_d = nc.declare_dram_parameter("onesr", [1, 128], dt.float32, isOutput=False)
    out_d = nc.declare_dram_parameter("out_t", [QW, S], dt.float32, isOutput=True)

    with tile.TileContext(nc) as tc:
        with (
            tc.tile_pool(name="const", bufs=1) as cpool,
            tc.tile_pool(name="qkv", bufs=1) as qkvpool,
            tc.tile_pool(name="att", bufs=1) as attpool,
            tc.tile_pool(name="dram", bufs=1, space="DRAM") as dpool,
        ):
            # ---- resident constants / weights ----
            wqt = cpool.tile([128, KD, QW], dt.bfloat16)
            wkt = cpool.tile([128, KD, HD], dt.bfloat16)
            wvt = cpool.tile([128, KD, HD], dt.bfloat16)
            cosd = cpool.tile([HD, S], dt.bfloat16)
            sind = cpool.tile([HD, S], dt.bfloat16)
            swapt = cpool.tile([HD, HD], dt.bfloat16)
            ident = cpool.tile([HD, HD], dt.bfloat16)
            dmask = cpool.tile([128, 128], dt.float32)
            onesc = cpool.tile([128, 1], dt.bfloat16)
            onesr = cpool.tile([1, 128], dt.float32)

            wqt_r = wqt_d[:, :].rearrange("(k p) n -> p k n", p=128)
            # split weight loads so the first matmuls can start early
            for kg in range(4):
                ksl = slice(kg * 8, (kg + 1) * 8)
                nc.sync.dma_start(wqt[:, ksl, :], wqt_r[:, ksl, :])
            nc.sync.dma_start(wkt[:], wkt_d[:, :].rearrange("(k p) n -> p k n", p=128))
            nc.sync.dma_start(wvt[:], wvt_d[:, :].rearrange("(k p) n -> p k n", p=128))
            nc.sync.dma_start(cosd[:], cosd_d[:, :])
            nc.sync.dma_start(sind[:], sind_d[:, :])
            nc.sync.dma_start(swapt[:], swapt_d[:, :])
            nc.sync.dma_start(ident[:], ident_d[:, :])
            nc.sync.dma_start(dmask[:], dmask_d[:, :])
            nc.sync.dma_start(onesc[:], onesc_d[:, :])
            nc.sync.dma_start(onesr[:], onesr_d[:, :])

            # ---- persistent activations ----
            qt = [qkvpool.tile([HD, S], dt.bfloat16, name=f"qt{h}", tag=f"qt{h}")
                  for h in range(HL)]
            kt = qkvpool.tile([HD, S], dt.bfloat16)
            vt = qkvpool.tile([HD, S], dt.bfloat16)
            vv = qkvpool.tile([128, NT, HD], dt.bfloat16)   # [t_part, ti, hd]
            att = [attpool.tile([HD, S], dt.bfloat16, name=f"att{h}", tag=f"att{h}")
                   for h in range(HL)]

            xt_r = xt_d[:, :].rearrange("(k p) s -> p k s", p=128)

            # ================= phase 1: projections + rope =================
            with (
                tc.tile_pool(name="xc", bufs=2) as xpool,
                tc.tile_pool(name="p1", bufs=3, space="PSUM") as pp1,
                tc.tile_pool(name="pr", bufs=2, space="PSUM") as ppr,
                tc.tile_pool(name="rtmp", bufs=3) as rtpool,
            ):
                for sc in range(NSC):
                    ssl = slice(sc * SC, (sc + 1) * SC)
                    xc = xpool.tile([128, KD, SC], dt.bfloat16)
                    for kg in range(4):
                        ksl = slice(kg * 8, (kg + 1) * 8)
                        nc.sync.dma_start(xc[:, ksl, :], xt_r[:, ksl, ssl])

                    # 4 Q heads (rope), K (rope), V (plain) — all [hd, s]
                    for hi in range(HL + 2):
                        ps = pp1.tile([128, SC], dt.float32)
                        for k in range(KD):
                            if hi < HL:
                                lhs = wqt[:, k, hi * HD:(hi + 1) * HD]
                            elif hi == HL:
                                lhs = wkt[:, k, :]
                            else:
                                lhs = wvt[:, k, :]
                            nc.tensor.matmul(ps[:], lhs, xc[:, k, :],
                                             start=(k == 0), stop=(k == KD - 1))
                        if hi == HL + 1:
                            nc.scalar.copy(vt[:, ssl], ps[:])
                            continue
                        # rope: out = q*cos + rot(q)*sin, rot via swap-matmul
                        qs = rtpool.tile([128, SC], dt.bfloat16, tag="ropeqs")
                        qc = rtpool.tile([128, SC], dt.bfloat16, tag="ropeqc")
                        nc.vector.tensor_mul(qs[:], ps[:], sind[:, ssl])
                        nc.vector.tensor_mul(qc[:], ps[:], cosd[:, ssl])
                        ps2 = ppr.tile([128, SC], dt.float32)
                        nc.tensor.matmul(ps2[:], swapt[:], qs[:], start=True, stop=False)
                        nc.tensor.matmul(ps2[:], ident[:], qc[:], start=False, stop=True)
                        dst = qt[hi] if hi < HL else kt
                        nc.scalar.copy(dst[:, ssl], ps2[:])

                    # V tiles in [t, hd] layout via DMA transpose
                    for vtile in range(4):
                        ti = sc * 4 + vtile
                        nc.sync.dma_start_transpose(
                            vv[:, ti, :], vt[:, ti * 128:(ti + 1) * 128])

            # ============ phase 2+3: attention, allgather, out-proj ============
            with (
                tc.tile_pool(name="wo", bufs=1) as wopool,
                tc.tile_pool(name="agc", bufs=2) as agpool,
                tc.tile_pool(name="st", bufs=3, space="PSUM") as stpool,
                tc.tile_pool(name="pv", bufs=2, space="PSUM") as pvpool,
                tc.tile_pool(name="rs", bufs=1, space="PSUM") as rspool,
                tc.tile_pool(name="p3", bufs=2, space="PSUM") as pp3,
                tc.tile_pool(name="pt", bufs=4) as ptpool,
                tc.tile_pool(name="ep", bufs=2) as eppool,
                tc.tile_pool(name="o3", bufs=3) as opool,
            ):
                wot = wopool.tile([128, KD, QW], dt.bfloat16)
                nc.sync.dma_start(wot[:], wot_d[:, :].rearrange("(k p) n -> p k n", p=128))
                ag_r = None
                for sc in range(NSC):
                    ssl = slice(sc * SC, (sc + 1) * SC)
                    n_t = sc * 4 + 4
                    for h in range(HL):
                        pv = pvpool.tile([128, SC], dt.float32)
                        rs = rspool.tile([1, SC], dt.float32)
                        for ti in range(n_t):
                            # valid columns: s >= ti*128; earlier ti's already
                            # wrote the full psum width (ti==0 has v == 0)
                            d_off = ti * 128 - sc * SC
                            v0 = max(d_off, 0)
                            vsl = slice(v0, SC)
                            qsl = slice(sc * SC + v0, (sc + 1) * SC)
                            st = stpool.tile([128, SC], dt.float32)
                            nc.tensor.matmul(st[:, vsl],
                                             kt[:, ti * 128:(ti + 1) * 128],
                                             qt[h][:, qsl], start=True, stop=True)
                            if d_off >= 0:
                                nc.vector.tensor_add(st[:, d_off:d_off + 128],
                                                     st[:, d_off:d_off + 128],
                                                     dmask[:])
                            pt = ptpool.tile([128, SC], dt.bfloat16)
                            nc.scalar.activation(pt[:, vsl], st[:, vsl],
                                                 mybir.ActivationFunctionType.Exp,
                                                 scale=SCALE)
                            nc.tensor.matmul(rs[:, vsl], onesc[:], pt[:, vsl],
                                             start=(ti == 0), stop=(ti == n_t - 1))
                            nc.tensor.matmul(pv[:, vsl], vv[:, ti, :], pt[:, vsl],
                                             start=(ti == 0), stop=(ti == n_t - 1))
                        # epilogue: normalize columns by 1/rowsum
                        rec = eppool.tile([1, SC], dt.float32, tag="rec")
                        nc.vector.reciprocal(rec[:], rs[:])
                        bc = pp3.tile([128, SC], dt.float32, tag="ps3")
                        nc.tensor.matmul(bc[:], onesr[:], rec[:], start=True, stop=True)
                        bcs = eppool.tile([128, SC], dt.float32, tag="bcs")
                        nc.scalar.copy(bcs[:], bc[:])
                        nc.vector.tensor_mul(att[h][:, ssl], pv[:], bcs[:])

                    # chunked AllGather of this s-chunk's attnT
                    ag_in = dpool.tile([QW, SC], dt.bfloat16,
                                       name=f"agi{sc}", tag=f"agi{sc}")
                    ag_out = dpool.tile([NCORES * QW, SC], dt.bfloat16,
                                        name=f"ago{sc}", tag=f"ago{sc}",
                                        addr_space="Shared")
                    for h in range(HL):
                        nc.sync.dma_start(ag_in[h * HD:(h + 1) * HD, :],
                                          att[h][:, ssl])
                    nc.gpsimd.collective_compute(
                        "AllGather",
                        mybir.AluOpType.bypass,
                        replica_groups=[list(range(NCORES))],
                        ins=[ag_in.opt()],
                        outs=[ag_out.opt()],
                    )
                    ag_r = ag_out[:, :].rearrange("(k p) s -> p k s", p=128)

                    # out-proj for this s-chunk
                    agcs = []
                    for kc in range(2):
                        agc = agpool.tile([128, KD // 2, SC], dt.bfloat16)
                        nc.sync.dma_start(agc[:], ag_r[:, kc * 16:(kc + 1) * 16, :])
                        agcs.append(agc)
                    for oc in range(4):
                        ps = pp3.tile([128, SC], dt.float32, tag="ps3")
                        for k in range(KD):
                            nc.tensor.matmul(ps[:], wot[:, k, oc * 128:(oc + 1) * 128],
                                             agcs[k // 16][:, k % 16, :],
                                             start=(k == 0), stop=(k == KD - 1))
                        ot = opool.tile([128, SC], dt.float32)
                        nc.vector.tensor_copy(ot[:], ps[:])
                        nc.sync.dma_start(out_d[oc * 128:(oc + 1) * 128, ssl], ot[:])
    if not nc.is_finalized():
        nc.finalize()
    return nc


_CACHE = {}


def _get_nc():
    if "nc" not in _CACHE:
        _CACHE["nc"] = _build_nc()
    return _CACHE["nc"]


def _prep_in_maps(x, wq, wk, wv, wo, freqs_cos, freqs_sin):
    xt = np.ascontiguousarray(x.reshape(S, D).T).astype(BF)
    cosd = np.repeat(np.asarray(freqs_cos, np.float32).T, 2, axis=0).astype(BF)
    sind = np.repeat(np.asarray(freqs_sin, np.float32).T, 2, axis=0).astype(BF)
    swapt = np.zeros((HD, HD), np.float32)
    for i in range(HD // 2):
        swapt[2 * i + 1, 2 * i] = -1.0
        swapt[2 * i, 2 * i + 1] = 1.0
    swapt = swapt.astype(BF)
    ident = np.eye(HD, dtype=np.float32).astype(BF)
    t_idx = np.arange(128)[:, None]
    s_idx = np.arange(128)[None, :]
    dmask = np.where(s_idx >= t_idx, 0.0, NEG).astype(np.float32)
    onesc = np.ones((128, 1), np.float32).astype(BF)
    onesr = np.ones((1, 128), np.float32)

    wq = np.asarray(wq, np.float32)
    wk = np.asarray(wk, np.float32)
    wv = np.asarray(wv, np.float32)
    wo = np.asarray(wo, np.float32)

    in_maps = []
    for c in range(NCORES):
        qsl = slice(QW * c, QW * (c + 1))
        ksl = slice(HD * c, HD * (c + 1))
        in_maps.append({
            "xt": xt,
            "wqt": np.ascontiguousarray(wq[qsl].T).astype(BF),
            "wkt": np.ascontiguousarray(wk[ksl].T).astype(BF),
            "wvt": np.ascontiguousarray(wv[ksl].T).astype(BF),
            "wot": np.ascontiguousarray(wo[qsl].T).astype(BF),
            "cosd": cosd, "sind": sind, "swapt": swapt, "ident": ident,
            "dmask": dmask, "onesc": onesc, "onesr": onesr,
        })
    return in_maps


def run(inputs, trace=False):
    from concourse.bass_utils import run_bass_kernel_spmd
    nc = _get_nc()
    in_maps = _prep_in_maps(
        inputs["x"], inputs["wq"], inputs["wk"], inputs["wv"], inputs["wo"],
        inputs["freqs_cos"], inputs["freqs_sin"])
    res = run_bass_kernel_spmd(nc, in_maps, core_ids=list(range(NCORES)),
                               trace=trace)
    shards = [np.asarray(res.results[c]["out_t"], np.float32)
              for c in range(NCORES)]
    full = np.concatenate(shards, axis=0)          # [4096, 2048]
    out = np.ascontiguousarray(full.T)[None]       # [1, 2048, 4096]
    return out.astype(np.float32), res


def kernel(**inputs):
    out, _ = run(inputs, trace=False)
    return out


# revision 13
# speedup vs baseline: 1.2284x; 1.0559x over previous
"""Distributed GQA attention kernel for 8 TRN2 NeuronCores.

Problem: B=1, S=2048, D=4096, H=32 q-heads, KV=8 kv-heads, HD=128.
  q = rope(x@wq.T), k = rope(x@wk.T), v = x@wv.T
  out = softmax(causal(q@k.T/sqrt(HD))) @ v @ wo.T

Sharding: tensor-parallel over heads. Core c owns q-heads 4c..4c+3 and
kv-head c. Device-side per core:
  phase 1: QT/KT (rope'd, [hd, s] layout) + VT projections; V tiles
           ([t, hd]) via DMA transpose
  phase 2: causal attention producing attnT chunks; software-pipelined:
           epilogues deferred one head, AllGather + out-proj deferred
           one s-chunk so they overlap the next chunk's attention.
Host side: layout prep (transposes, bf16 cast, rope tables) + final
concat/transpose of the 8 out.T slices.
"""

import math
import numpy as np
import ml_dtypes

BF = ml_dtypes.bfloat16

B, S, D = 1, 2048, 4096
H, KV, HD = 32, 8, 128
NCORES = 8
HL = H // NCORES            # 4 local q heads
QW = HL * HD                # 512 local q width
SC = 512                    # s-chunk width
NSC = S // SC               # 4 s-chunks
KD = 32                     # d-dim k-tiles (4096/128)
NT = S // 128               # 16 t-tiles
SCALE = 1.0 / math.sqrt(HD)
NEG = -30000.0


def _build_nc():
    import concourse.bass as bass
    import concourse.mybir as mybir
    from concourse import bacc, tile

    dt = mybir.dt
    nc = bacc.Bacc()

    xt_d = nc.declare_dram_parameter("xt", [D, S], dt.bfloat16, isOutput=False)
    wqt_d = nc.declare_dram_parameter("wqt", [D, QW], dt.bfloat16, isOutput=False)
    wkt_d = nc.declare_dram_parameter("wkt", [D, HD], dt.bfloat16, isOutput=False)
    wvt_d = nc.declare_dram_parameter("wvt", [D, HD], dt.bfloat16, isOutput=False)
    wot_d = nc.declare_dram_parameter("wot", [D, QW], dt.bfloat16, isOutput=False)
    cosd_d = nc.declare_dram_parameter("cosd", [HD, S], dt.bfloat16, isOutput=False)
    sind_d = nc.declare_dram_parameter("sind", [HD, S], dt.bfloat16, isOutput=False)
    swapt_d = nc.declare_dram_parameter("swapt", [HD, HD], dt.bfloat16, isOutput=False)
    ident_d = nc.declare_dram_parameter("ident", [HD, HD], dt.bfloat16, isOutput=False)
    dmask_d = nc.declare_dram_parameter("dmask", [128, 128], dt.float32, isOutput=False)
    onesc_d = nc.declare_dram_parameter("onesc", [128, 1], dt.bfloat16, isOutput=False)
    onesr_d = nc.declare_dram_parameter("onesr", [1, 128], dt.float32, isOutput=False)
    out_d = nc.declare_dram_parameter("out_t", [QW, S], dt.float32, isOutput=True)

    with tile.TileContext(nc) as tc:
        with (
            tc.tile_pool(name="const", bufs=1) as cpool,
            tc.tile_pool(name="qkv", bufs=1) as qkvpool,
            tc.tile_pool(name="att", bufs=1) as attpool,
            tc.tile_pool(name="dram", bufs=1, space="DRAM") as dpool,
        ):
            # ---- small resident constants ----
            cosd = cpool.tile([HD, S], dt.bfloat16)
            sind = cpool.tile([HD, S], dt.bfloat16)
            swapt = cpool.tile([HD, HD], dt.bfloat16)
            ident = cpool.tile([HD, HD], dt.bfloat16)
            dmask = cpool.tile([128, 128], dt.float32)
            onesc = cpool.tile([128, 1], dt.bfloat16)
            onesr = cpool.tile([1, 128], dt.float32)
            nc.sync.dma_start(cosd[:], cosd_d[:, :])
            nc.sync.dma_start(sind[:], sind_d[:, :])
            nc.sync.dma_start(swapt[:], swapt_d[:, :])
            nc.sync.dma_start(ident[:], ident_d[:, :])
            nc.sync.dma_start(dmask[:], dmask_d[:, :])
            nc.sync.dma_start(onesc[:], onesc_d[:, :])
            nc.sync.dma_start(onesr[:], onesr_d[:, :])

            # ---- persistent activations ----
            qt = [qkvpool.tile([HD, S], dt.bfloat16, name=f"qt{h}", tag=f"qt{h}")
                  for h in range(HL)]
            kt = qkvpool.tile([HD, S], dt.bfloat16)
            vv = qkvpool.tile([128, NT, HD], dt.bfloat16)   # [t_part, ti, hd]
            att = [attpool.tile([HD, S], dt.bfloat16, name=f"att{h}", tag=f"att{h}")
                   for h in range(HL)]

            xt_r = xt_d[:, :].rearrange("(k p) s -> p k s", p=128)

            # ================= phase 1: projections + rope =================
            with (
                tc.tile_pool(name="w1", bufs=1) as wpool,
                tc.tile_pool(name="xc", bufs=2) as xpool,
                tc.tile_pool(name="p1", bufs=3, space="PSUM") as pp1,
                tc.tile_pool(name="pr", bufs=2, space="PSUM") as ppr,
                tc.tile_pool(name="rtmp", bufs=3) as rtpool,
            ):
                wqt = wpool.tile([128, KD, QW], dt.bfloat16)
                wkt = wpool.tile([128, KD, HD], dt.bfloat16)
                wvt = wpool.tile([128, KD, HD], dt.bfloat16)
                vt = wpool.tile([HD, S], dt.bfloat16)
                wqt_r = wqt_d[:, :].rearrange("(k p) n -> p k n", p=128)
                # split weight loads so the first matmuls can start early
                for kg in range(4):
                    ksl = slice(kg * 8, (kg + 1) * 8)
                    nc.sync.dma_start(wqt[:, ksl, :], wqt_r[:, ksl, :])
                nc.sync.dma_start(
                    wkt[:], wkt_d[:, :].rearrange("(k p) n -> p k n", p=128))
                nc.sync.dma_start(
                    wvt[:], wvt_d[:, :].rearrange("(k p) n -> p k n", p=128))

                for sc in range(NSC):
                    ssl = slice(sc * SC, (sc + 1) * SC)
                    xc = xpool.tile([128, KD, SC], dt.bfloat16)
                    for kg in range(4):
                        ksl = slice(kg * 8, (kg + 1) * 8)
                        nc.sync.dma_start(xc[:, ksl, :], xt_r[:, ksl, ssl])

                    # 4 Q heads (rope), K (rope), V (plain) — all [hd, s]
                    for hi in range(HL + 2):
                        ps = pp1.tile([128, SC], dt.float32)
                        for k in range(KD):
                            if hi < HL:
                                lhs = wqt[:, k, hi * HD:(hi + 1) * HD]
                            elif hi == HL:
                                lhs = wkt[:, k, :]
                            else:
                                lhs = wvt[:, k, :]
                            nc.tensor.matmul(ps[:], lhs, xc[:, k, :],
                                             start=(k == 0), stop=(k == KD - 1))
                        if hi == HL + 1:
                            nc.scalar.copy(vt[:, ssl], ps[:])
                            continue
                        # rope: out = q*cos + rot(q)*sin, rot via swap-matmul
                        qs = rtpool.tile([128, SC], dt.bfloat16, tag="ropeqs")
                        qc = rtpool.tile([128, SC], dt.bfloat16, tag="ropeqc")
                        nc.vector.tensor_mul(qs[:], ps[:], sind[:, ssl])
                        nc.vector.tensor_mul(qc[:], ps[:], cosd[:, ssl])
                        ps2 = ppr.tile([128, SC], dt.float32)
                        nc.tensor.matmul(ps2[:], swapt[:], qs[:], start=True, stop=False)
                        nc.tensor.matmul(ps2[:], ident[:], qc[:], start=False, stop=True)
                        dst = qt[hi] if hi < HL else kt
                        nc.scalar.copy(dst[:, ssl], ps2[:])

                    # V tiles in [t, hd] layout via DMA transpose
                    for vtile in range(4):
                        ti = sc * 4 + vtile
                        nc.sync.dma_start_transpose(
                            vv[:, ti, :], vt[:, ti * 128:(ti + 1) * 128])

            # ============ phase 2+3: attention, allgather, out-proj ============
            with (
                tc.tile_pool(name="wo", bufs=1) as wopool,
                tc.tile_pool(name="agc", bufs=2) as agpool,
                tc.tile_pool(name="st", bufs=2, space="PSUM") as stpool,
                tc.tile_pool(name="pv", bufs=2, space="PSUM") as pvpool,
                tc.tile_pool(name="rs", bufs=2, space="PSUM") as rspool,
                tc.tile_pool(name="p3", bufs=2, space="PSUM") as pp3,
                tc.tile_pool(name="pt", bufs=4) as ptpool,
                tc.tile_pool(name="ep", bufs=2) as eppool,
                tc.tile_pool(name="o3", bufs=3) as opool,
            ):
                wot = wopool.tile([128, KD, QW], dt.bfloat16)
                nc.sync.dma_start(
                    wot[:], wot_d[:, :].rearrange("(k p) n -> p k n", p=128))

                def epilogue(sc, h, pv, rs):
                    # normalize columns of attnT by 1/rowsum
                    ssl = slice(sc * SC, (sc + 1) * SC)
                    rec = eppool.tile([1, SC], dt.float32, tag="rec")
                    nc.vector.reciprocal(rec[:], rs[:])
                    bc = pp3.tile([128, SC], dt.float32, tag="ps3")
                    nc.tensor.matmul(bc[:], onesr[:], rec[:], start=True, stop=True)
                    bcs = eppool.tile([128, SC], dt.float32, tag="bcs")
                    nc.scalar.copy(bcs[:], bc[:])
                    nc.vector.tensor_mul(att[h][:, ssl], pv[:], bcs[:])

                def allgather(sc):
                    ssl = slice(sc * SC, (sc + 1) * SC)
                    ag_in = dpool.tile([QW, SC], dt.bfloat16,
                                       name=f"agi{sc}", tag=f"agi{sc}")
                    ag_out = dpool.tile([NCORES * QW, SC], dt.bfloat16,
                                        name=f"ago{sc}", tag=f"ago{sc}",
                                        addr_space="Shared")
                    for h in range(HL):
                        nc.sync.dma_start(ag_in[h * HD:(h + 1) * HD, :],
                                          att[h][:, ssl])
                    nc.gpsimd.collective_compute(
                        "AllGather",
                        mybir.AluOpType.bypass,
                        replica_groups=[list(range(NCORES))],
                        ins=[ag_in.opt()],
                        outs=[ag_out.opt()],
                    )
                    return ag_out

                def outproj(sc, ag_out):
                    ssl = slice(sc * SC, (sc + 1) * SC)
                    ag_r = ag_out[:, :].rearrange("(k p) s -> p k s", p=128)
                    agc = agpool.tile([128, KD, SC], dt.bfloat16)
                    nc.sync.dma_start(agc[:], ag_r[:, :, :])
                    for oc in range(4):
                        ps = pp3.tile([128, SC], dt.float32, tag="ps3")
                        for k in range(KD):
                            nc.tensor.matmul(
                                ps[:], wot[:, k, oc * 128:(oc + 1) * 128],
                                agc[:, k, :],
                                start=(k == 0), stop=(k == KD - 1))
                        ot = opool.tile([128, SC], dt.float32)
                        nc.vector.tensor_copy(ot[:], ps[:])
                        nc.sync.dma_start(out_d[oc * 128:(oc + 1) * 128, ssl], ot[:])

                pending_ep = None   # (sc, h, pv, rs)
                ag_prev = None      # (sc, ag_out) awaiting outproj
                for sc in range(NSC):
                    ssl = slice(sc * SC, (sc + 1) * SC)
                    n_t = sc * 4 + 4
                    for h in range(HL):
                        pv = pvpool.tile([128, SC], dt.float32)
                        rs = rspool.tile([1, SC], dt.float32)
                        for ti in range(n_t):
                            # valid columns: s >= ti*128 (ti==0 covers all)
                            d_off = ti * 128 - sc * SC
                            v0 = max(d_off, 0)
                            vsl = slice(v0, SC)
                            qcl = slice(sc * SC + v0, (sc + 1) * SC)
                            st = stpool.tile([128, SC], dt.float32)
                            nc.tensor.matmul(st[:, vsl],
                                             kt[:, ti * 128:(ti + 1) * 128],
                                             qt[h][:, qcl], start=True, stop=True)
                            if d_off >= 0:
                                nc.vector.tensor_add(st[:, d_off:d_off + 128],
                                                     st[:, d_off:d_off + 128],
                                                     dmask[:])
                            pt = ptpool.tile([128, SC], dt.bfloat16)
                            nc.scalar.activation(pt[:, vsl], st[:, vsl],
                                                 mybir.ActivationFunctionType.Exp,
                                                 scale=SCALE)
                            nc.tensor.matmul(rs[:, vsl], onesc[:], pt[:, vsl],
                                             start=(ti == 0), stop=(ti == n_t - 1))
                            nc.tensor.matmul(pv[:, vsl], vv[:, ti, :], pt[:, vsl],
                                             start=(ti == 0), stop=(ti == n_t - 1))
                        # deferred work, placed after this head's matmul stream
                        if pending_ep is not None:
                            epilogue(*pending_ep)
                            pending_ep = None
                        if h == 0 and sc > 0:
                            # previous chunk: all 4 epilogues done -> gather it
                            ag_prev = (sc - 1, allgather(sc - 1))
                        if h == 2 and ag_prev is not None:
                            outproj(ag_prev[0], ag_prev[1])
                            ag_prev = None
                        pending_ep = (sc, h, pv, rs)
                # tail
                epilogue(*pending_ep)
                ag3 = allgather(NSC - 1)
                if ag_prev is not None:
                    outproj(ag_prev[0], ag_prev[1])
                outproj(NSC - 1, ag3)
    if not nc.is_finalized():
        nc.finalize()
    return nc


_CACHE = {}


def _get_nc():
    if "nc" not in _CACHE:
        _CACHE["nc"] = _build_nc()
    return _CACHE["nc"]


def _prep_in_maps(x, wq, wk, wv, wo, freqs_cos, freqs_sin):
    xt = np.ascontiguousarray(x.reshape(S, D).T).astype(BF)
    cosd = np.repeat(np.asarray(freqs_cos, np.float32).T, 2, axis=0).astype(BF)
    sind = np.repeat(np.asarray(freqs_sin, np.float32).T, 2, axis=0).astype(BF)
    swapt = np.zeros((HD, HD), np.float32)
    for i in range(HD // 2):
        swapt[2 * i + 1, 2 * i] = -1.0
        swapt[2 * i, 2 * i + 1] = 1.0
    swapt = swapt.astype(BF)
    ident = np.eye(HD, dtype=np.float32).astype(BF)
    t_idx = np.arange(128)[:, None]
    s_idx = np.arange(128)[None, :]
    dmask = np.where(s_idx >= t_idx, 0.0, NEG).astype(np.float32)
    onesc = np.ones((128, 1), np.float32).astype(BF)
    onesr = np.ones((1, 128), np.float32)

    wq = np.asarray(wq, np.float32)
    wk = np.asarray(wk, np.float32)
    wv = np.asarray(wv, np.float32)
    wo = np.asarray(wo, np.float32)

    in_maps = []
    for c in range(NCORES):
        qsl = slice(QW * c, QW * (c + 1))
        ksl = slice(HD * c, HD * (c + 1))
        in_maps.append({
            "xt": xt,
            "wqt": np.ascontiguousarray(wq[qsl].T).astype(BF),
            "wkt": np.ascontiguousarray(wk[ksl].T).astype(BF),
            "wvt": np.ascontiguousarray(wv[ksl].T).astype(BF),
            "wot": np.ascontiguousarray(wo[qsl].T).astype(BF),
            "cosd": cosd, "sind": sind, "swapt": swapt, "ident": ident,
            "dmask": dmask, "onesc": onesc, "onesr": onesr,
        })
    return in_maps


def run(inputs, trace=False):
    from concourse.bass_utils import run_bass_kernel_spmd
    nc = _get_nc()
    in_maps = _prep_in_maps(
        inputs["x"], inputs["wq"], inputs["wk"], inputs["wv"], inputs["wo"],
        inputs["freqs_cos"], inputs["freqs_sin"])
    res = run_bass_kernel_spmd(nc, in_maps, core_ids=list(range(NCORES)),
                               trace=trace)
    shards = [np.asarray(res.results[c]["out_t"], np.float32)
              for c in range(NCORES)]
    full = np.concatenate(shards, axis=0)          # [4096, 2048]
    out = np.ascontiguousarray(full.T)[None]       # [1, 2048, 4096]
    return out.astype(np.float32), res


def kernel(**inputs):
    out, _ = run(inputs, trace=False)
    return out


# revision 16
# speedup vs baseline: 1.3302x; 1.0829x over previous
"""Distributed GQA attention kernel for 8 TRN2 NeuronCores.

Problem: B=1, S=2048, D=4096, H=32 q-heads, KV=8 kv-heads, HD=128.
  q = rope(x@wq.T), k = rope(x@wk.T), v = x@wv.T
  out = softmax(causal(q@k.T/sqrt(HD))) @ v @ wo.T

Sharding: tensor-parallel over heads. Core c owns q-heads 4c..4c+3 and
kv-head c. Device-side per core:
  phase 1: QT/KT (rope'd, [hd, s] layout) + VT projections; V tiles
           ([t, hd]) via DMA transpose
  phase 2: causal attention producing attnT chunks; software-pipelined:
           epilogues deferred one head, AllGather + out-proj deferred
           one s-chunk so they overlap the next chunk's attention.
Host side: layout prep (transposes, bf16 cast, rope tables) + final
concat/transpose of the 8 out.T slices.
"""

import math
import numpy as np
import ml_dtypes

BF = ml_dtypes.bfloat16

B, S, D = 1, 2048, 4096
H, KV, HD = 32, 8, 128
NCORES = 8
HL = H // NCORES            # 4 local q heads
QW = HL * HD                # 512 local q width
SC = 512                    # s-chunk width
NSC = S // SC               # 4 s-chunks
KD = 32                     # d-dim k-tiles (4096/128)
NT = S // 128               # 16 t-tiles
SCALE = 1.0 / math.sqrt(HD)
NEG = -30000.0


def _build_nc():
    import concourse.bass as bass
    import concourse.mybir as mybir
    from concourse import bacc, tile

    dt = mybir.dt
    nc = bacc.Bacc()

    xt_d = nc.declare_dram_parameter("xt", [D, S], dt.bfloat16, isOutput=False)
    wqt_d = nc.declare_dram_parameter("wqt", [D, QW], dt.bfloat16, isOutput=False)
    wkt_d = nc.declare_dram_parameter("wkt", [D, HD], dt.bfloat16, isOutput=False)
    wvt_d = nc.declare_dram_parameter("wvt", [D, HD], dt.bfloat16, isOutput=False)
    wot_d = nc.declare_dram_parameter("wot", [D, QW], dt.bfloat16, isOutput=False)
    cosd_d = nc.declare_dram_parameter("cosd", [HD, S], dt.bfloat16, isOutput=False)
    sind_d = nc.declare_dram_parameter("sind", [HD, S], dt.bfloat16, isOutput=False)
    swapt_d = nc.declare_dram_parameter("swapt", [HD, HD], dt.bfloat16, isOutput=False)
    ident_d = nc.declare_dram_parameter("ident", [HD, HD], dt.bfloat16, isOutput=False)
    dmask_d = nc.declare_dram_parameter("dmask", [128, 128], dt.float32, isOutput=False)
    onesc_d = nc.declare_dram_parameter("onesc", [128, 1], dt.bfloat16, isOutput=False)
    onesr_d = nc.declare_dram_parameter("onesr", [1, 128], dt.float32, isOutput=False)
    out_d = nc.declare_dram_parameter("out_t", [QW, S], dt.float32, isOutput=True)

    with tile.TileContext(nc) as tc:
        with (
            tc.tile_pool(name="const", bufs=1) as cpool,
            tc.tile_pool(name="qkv", bufs=1) as qkvpool,
            tc.tile_pool(name="att", bufs=1) as attpool,
            tc.tile_pool(name="dram", bufs=1, space="DRAM") as dpool,
        ):
            # ---- small resident constants ----
            cosd = cpool.tile([HD, S], dt.bfloat16)
            sind = cpool.tile([HD, S], dt.bfloat16)
            swapt = cpool.tile([HD, HD], dt.bfloat16)
            ident = cpool.tile([HD, HD], dt.bfloat16)
            dmask = cpool.tile([128, 128], dt.float32)
            onesc = cpool.tile([128, 1], dt.bfloat16)
            onesr = cpool.tile([1, 128], dt.float32)
            nc.sync.dma_start(cosd[:], cosd_d[:, :])
            nc.sync.dma_start(sind[:], sind_d[:, :])
            nc.sync.dma_start(swapt[:], swapt_d[:, :])
            nc.sync.dma_start(ident[:], ident_d[:, :])
            nc.sync.dma_start(dmask[:], dmask_d[:, :])
            nc.sync.dma_start(onesc[:], onesc_d[:, :])
            nc.sync.dma_start(onesr[:], onesr_d[:, :])

            # ---- persistent activations ----
            qt = [qkvpool.tile([HD, S], dt.bfloat16, name=f"qt{h}", tag=f"qt{h}")
                  for h in range(HL)]
            kt = qkvpool.tile([HD, S], dt.bfloat16)
            vv = qkvpool.tile([128, NT, HD], dt.bfloat16)   # [t_part, ti, hd]
            att = [attpool.tile([HD, S], dt.bfloat16, name=f"att{h}", tag=f"att{h}")
                   for h in range(HL)]

            xt_r = xt_d[:, :].rearrange("(k p) s -> p k s", p=128)

            # ================= phase 1: projections + rope =================
            with (
                tc.tile_pool(name="w1", bufs=1) as wpool,
                tc.tile_pool(name="xc", bufs=2) as xpool,
                tc.tile_pool(name="p1", bufs=3, space="PSUM") as pp1,
                tc.tile_pool(name="pr", bufs=2, space="PSUM") as ppr,
                tc.tile_pool(name="rtmp", bufs=3) as rtpool,
            ):
                wqt = wpool.tile([128, KD, QW], dt.bfloat16)
                wkt = wpool.tile([128, KD, HD], dt.bfloat16)
                wvt = wpool.tile([128, KD, HD], dt.bfloat16)
                vt = wpool.tile([HD, S], dt.bfloat16)
                wqt_r = wqt_d[:, :].rearrange("(k p) n -> p k n", p=128)
                # interleave first x-chunk with weight loads so the first
                # matmuls can start as early as possible
                xc0 = xpool.tile([128, KD, SC], dt.bfloat16, tag="xc")
                for kg in range(4):
                    ksl = slice(kg * 8, (kg + 1) * 8)
                    nc.sync.dma_start(xc0[:, ksl, :], xt_r[:, ksl, 0:SC])
                    nc.sync.dma_start(wqt[:, ksl, :], wqt_r[:, ksl, :])
                nc.sync.dma_start(
                    wkt[:], wkt_d[:, :].rearrange("(k p) n -> p k n", p=128))
                nc.sync.dma_start(
                    wvt[:], wvt_d[:, :].rearrange("(k p) n -> p k n", p=128))

                for sc in range(NSC):
                    ssl = slice(sc * SC, (sc + 1) * SC)
                    if sc == 0:
                        xc = xc0
                    else:
                        xc = xpool.tile([128, KD, SC], dt.bfloat16, tag="xc")
                        for kg in range(4):
                            ksl = slice(kg * 8, (kg + 1) * 8)
                            nc.sync.dma_start(xc[:, ksl, :], xt_r[:, ksl, ssl])

                    # 4 Q heads (rope), K (rope), V (plain) — all [hd, s]
                    for hi in range(HL + 2):
                        ps = pp1.tile([128, SC], dt.float32)
                        for k in range(KD):
                            if hi < HL:
                                lhs = wqt[:, k, hi * HD:(hi + 1) * HD]
                            elif hi == HL:
                                lhs = wkt[:, k, :]
                            else:
                                lhs = wvt[:, k, :]
                            nc.tensor.matmul(ps[:], lhs, xc[:, k, :],
                                             start=(k == 0), stop=(k == KD - 1))
                        if hi == HL + 1:
                            nc.scalar.copy(vt[:, ssl], ps[:])
                            continue
                        # rope: out = q*cos + rot(q)*sin, rot via swap-matmul
                        qs = rtpool.tile([128, SC], dt.bfloat16, tag="ropeqs")
                        qc = rtpool.tile([128, SC], dt.bfloat16, tag="ropeqc")
                        nc.vector.tensor_mul(qs[:], ps[:], sind[:, ssl])
                        nc.vector.tensor_mul(qc[:], ps[:], cosd[:, ssl])
                        ps2 = ppr.tile([128, SC], dt.float32)
                        nc.tensor.matmul(ps2[:], swapt[:], qs[:], start=True, stop=False)
                        nc.tensor.matmul(ps2[:], ident[:], qc[:], start=False, stop=True)
                        dst = qt[hi] if hi < HL else kt
                        nc.scalar.copy(dst[:, ssl], ps2[:])

                    # V tiles in [t, hd] layout via DMA transpose
                    for vtile in range(4):
                        ti = sc * 4 + vtile
                        nc.sync.dma_start_transpose(
                            vv[:, ti, :], vt[:, ti * 128:(ti + 1) * 128])

            # ============ phase 2+3: attention, allgather, out-proj ============
            with (
                tc.tile_pool(name="wo", bufs=1) as wopool,
                tc.tile_pool(name="agc", bufs=2) as agpool,
                tc.tile_pool(name="st", bufs=2, space="PSUM") as stpool,
                tc.tile_pool(name="pv", bufs=2, space="PSUM") as pvpool,
                tc.tile_pool(name="rs", bufs=2, space="PSUM") as rspool,
                tc.tile_pool(name="p3", bufs=2, space="PSUM") as pp3,
                tc.tile_pool(name="pt", bufs=4) as ptpool,
                tc.tile_pool(name="ep", bufs=2) as eppool,
                tc.tile_pool(name="o3", bufs=3) as opool,
            ):
                wot = wopool.tile([128, KD, QW], dt.bfloat16)
                nc.sync.dma_start(
                    wot[:], wot_d[:, :].rearrange("(k p) n -> p k n", p=128))

                def epilogue(sc, h, pv, rs):
                    # normalize columns of attnT by 1/rowsum (no PE, no ACT)
                    ssl = slice(sc * SC, (sc + 1) * SC)
                    rec = eppool.tile([1, SC], dt.float32, tag="rec")
                    nc.vector.reciprocal(rec[:], rs[:])
                    bcs = eppool.tile([128, SC], dt.float32, tag="bcs")
                    nc.gpsimd.partition_broadcast(bcs[:], rec[0:1, :])
                    nc.vector.tensor_mul(att[h][:, ssl], pv[:], bcs[:])

                def allgather_half(sc, half):
                    # gather this core's head pair (2*half, 2*half+1):
                    # out block r covers global i-tiles {4r+2*half, 4r+2*half+1}
                    ssl = slice(sc * SC, (sc + 1) * SC)
                    ag_in = dpool.tile([2 * HD, SC], dt.bfloat16,
                                       name=f"agi{sc}{half}", tag=f"agi{sc}{half}")
                    ag_out = dpool.tile([NCORES * 2 * HD, SC], dt.bfloat16,
                                        name=f"ago{sc}{half}", tag=f"ago{sc}{half}",
                                        addr_space="Shared")
                    for hh in range(2):
                        h = 2 * half + hh
                        nc.sync.dma_start(ag_in[hh * HD:(hh + 1) * HD, :],
                                          att[h][:, ssl])
                    nc.gpsimd.collective_compute(
                        "AllGather",
                        mybir.AluOpType.bypass,
                        replica_groups=[list(range(NCORES))],
                        ins=[ag_in.opt()],
                        outs=[ag_out.opt()],
                    )
                    return ag_out

                def outproj(sc, ag_a, ag_b):
                    ssl = slice(sc * SC, (sc + 1) * SC)
                    agcs = []
                    for ag in (ag_a, ag_b):
                        ag_r = ag[:, :].rearrange("(m p) s -> p m s", p=128)
                        agc = agpool.tile([128, NT, SC], dt.bfloat16, tag="agc")
                        nc.sync.dma_start(agc[:], ag_r[:, :, :])
                        agcs.append(agc)
                    for oc in range(4):
                        ps = pp3.tile([128, SC], dt.float32, tag="ps3")
                        for half in range(2):
                            for m in range(NT):
                                kg = (m // 2) * 4 + (m % 2) + 2 * half
                                nc.tensor.matmul(
                                    ps[:], wot[:, kg, oc * 128:(oc + 1) * 128],
                                    agcs[half][:, m, :],
                                    start=(half == 0 and m == 0),
                                    stop=(half == 1 and m == NT - 1))
                        ot = opool.tile([128, SC], dt.float32)
                        nc.vector.tensor_copy(ot[:], ps[:])
                        nc.sync.dma_start(out_d[oc * 128:(oc + 1) * 128, ssl], ot[:])

                pending_op = None   # (sc, ag_a, ag_b) awaiting outproj
                ag_a_cur = None
                for sc in range(NSC):
                    n_t = sc * 4 + 4
                    for h in range(HL):
                        pv = pvpool.tile([128, SC], dt.float32)
                        rs = rspool.tile([1, SC], dt.float32)
                        for ti in range(n_t):
                            # valid columns: s >= ti*128 (ti==0 covers all)
                            d_off = ti * 128 - sc * SC
                            v0 = max(d_off, 0)
                            vsl = slice(v0, SC)
                            qcl = slice(sc * SC + v0, (sc + 1) * SC)
                            st = stpool.tile([128, SC], dt.float32)
                            nc.tensor.matmul(st[:, vsl],
                                             kt[:, ti * 128:(ti + 1) * 128],
                                             qt[h][:, qcl], start=True, stop=True)
                            if d_off >= 0:
                                nc.vector.tensor_add(st[:, d_off:d_off + 128],
                                                     st[:, d_off:d_off + 128],
                                                     dmask[:])
                            pt = ptpool.tile([128, SC], dt.bfloat16)
                            nc.scalar.activation(pt[:, vsl], st[:, vsl],
                                                 mybir.ActivationFunctionType.Exp,
                                                 scale=SCALE)
                            nc.tensor.matmul(rs[:, vsl], onesc[:], pt[:, vsl],
                                             start=(ti == 0), stop=(ti == n_t - 1))
                            nc.tensor.matmul(pv[:, vsl], vv[:, ti, :], pt[:, vsl],
                                             start=(ti == 0), stop=(ti == n_t - 1))
                        epilogue(sc, h, pv, rs)
                        if h == 1:
                            ag_a_cur = allgather_half(sc, 0)
                        if h == 2 and pending_op is not None:
                            outproj(*pending_op)
                            pending_op = None
                    ag_b_cur = allgather_half(sc, 1)
                    pending_op = (sc, ag_a_cur, ag_b_cur)
                outproj(*pending_op)
    if not nc.is_finalized():
        nc.finalize()
    return nc


_CACHE = {}


def _get_nc():
    if "nc" not in _CACHE:
        _CACHE["nc"] = _build_nc()
    return _CACHE["nc"]


def _prep_in_maps(x, wq, wk, wv, wo, freqs_cos, freqs_sin):
    xt = np.ascontiguousarray(x.reshape(S, D).T).astype(BF)
    cosd = np.repeat(np.asarray(freqs_cos, np.float32).T, 2, axis=0).astype(BF)
    sind = np.repeat(np.asarray(freqs_sin, np.float32).T, 2, axis=0).astype(BF)
    swapt = np.zeros((HD, HD), np.float32)
    for i in range(HD // 2):
        swapt[2 * i + 1, 2 * i] = -1.0
        swapt[2 * i, 2 * i + 1] = 1.0
    swapt = swapt.astype(BF)
    ident = np.eye(HD, dtype=np.float32).astype(BF)
    t_idx = np.arange(128)[:, None]
    s_idx = np.arange(128)[None, :]
    dmask = np.where(s_idx >= t_idx, 0.0, NEG).astype(np.float32)
    onesc = np.ones((128, 1), np.float32).astype(BF)
    onesr = np.ones((1, 128), np.float32)

    wq = np.asarray(wq, np.float32)
    wk = np.asarray(wk, np.float32)
    wv = np.asarray(wv, np.float32)
    wo = np.asarray(wo, np.float32)

    in_maps = []
    for c in range(NCORES):
        qsl = slice(QW * c, QW * (c + 1))
        ksl = slice(HD * c, HD * (c + 1))
        in_maps.append({
            "xt": xt,
            "wqt": np.ascontiguousarray(wq[qsl].T).astype(BF),
            "wkt": np.ascontiguousarray(wk[ksl].T).astype(BF),
            "wvt": np.ascontiguousarray(wv[ksl].T).astype(BF),
            "wot": np.ascontiguousarray(wo[qsl].T).astype(BF),
            "cosd": cosd, "sind": sind, "swapt": swapt, "ident": ident,
            "dmask": dmask, "onesc": onesc, "onesr": onesr,
        })
    return in_maps


def run(inputs, trace=False):
    from concourse.bass_utils import run_bass_kernel_spmd
    nc = _get_nc()
    in_maps = _prep_in_maps(
        inputs["x"], inputs["wq"], inputs["wk"], inputs["wv"], inputs["wo"],
        inputs["freqs_cos"], inputs["freqs_sin"])
    res = run_bass_kernel_spmd(nc, in_maps, core_ids=list(range(NCORES)),
                               trace=trace)
    shards = [np.asarray(res.results[c]["out_t"], np.float32)
              for c in range(NCORES)]
    full = np.concatenate(shards, axis=0)          # [4096, 2048]
    out = np.ascontiguousarray(full.T)[None]       # [1, 2048, 4096]
    return out.astype(np.float32), res


def kernel(**inputs):
    out, _ = run(inputs, trace=False)
    return out


# revision 20
# speedup vs baseline: 1.3518x; 1.0162x over previous
"""Distributed GQA attention kernel for 8 TRN2 NeuronCores.

Problem: B=1, S=2048, D=4096, H=32 q-heads, KV=8 kv-heads, HD=128.
  q = rope(x@wq.T), k = rope(x@wk.T), v = x@wv.T
  out = softmax(causal(q@k.T/sqrt(HD))) @ v @ wo.T

Sharding: tensor-parallel over heads. Core c owns q-heads 4c..4c+3 and
kv-head c. Device-side per core:
  phase 1: QT/KT (rope'd, [hd, s] layout) + VT projections; V tiles
           ([t, hd]) via DMA transpose
  phase 2: causal attention producing attnT chunks; software-pipelined:
           epilogues deferred one head, AllGather + out-proj deferred
           one s-chunk so they overlap the next chunk's attention.
Host side: layout prep (transposes, bf16 cast, rope tables) + final
concat/transpose of the 8 out.T slices.
"""

import math
import numpy as np
import ml_dtypes

BF = ml_dtypes.bfloat16

B, S, D = 1, 2048, 4096
H, KV, HD = 32, 8, 128
NCORES = 8
HL = H // NCORES            # 4 local q heads
QW = HL * HD                # 512 local q width
SC = 512                    # s-chunk width
NSC = S // SC               # 4 s-chunks
KD = 32                     # d-dim k-tiles (4096/128)
NT = S // 128               # 16 t-tiles
SCALE = 1.0 / math.sqrt(HD)
NEG = -30000.0


def _build_nc():
    import concourse.bass as bass
    import concourse.mybir as mybir
    from concourse import bacc, tile

    dt = mybir.dt
    nc = bacc.Bacc()

    xt_d = nc.declare_dram_parameter("xt", [D, S], dt.bfloat16, isOutput=False)
    wqt_d = nc.declare_dram_parameter("wqt", [D, QW], dt.bfloat16, isOutput=False)
    wkt_d = nc.declare_dram_parameter("wkt", [D, HD], dt.bfloat16, isOutput=False)
    wvt_d = nc.declare_dram_parameter("wvt", [D, HD], dt.bfloat16, isOutput=False)
    wot_d = nc.declare_dram_parameter("wot", [D, QW], dt.bfloat16, isOutput=False)
    cosd_d = nc.declare_dram_parameter("cosd", [HD, S], dt.bfloat16, isOutput=False)
    sind_d = nc.declare_dram_parameter("sind", [HD, S], dt.bfloat16, isOutput=False)
    swapt_d = nc.declare_dram_parameter("swapt", [HD, HD], dt.bfloat16, isOutput=False)
    ident_d = nc.declare_dram_parameter("ident", [HD, HD], dt.bfloat16, isOutput=False)
    dmask_d = nc.declare_dram_parameter("dmask", [128, 128], dt.float32, isOutput=False)
    onesc_d = nc.declare_dram_parameter("onesc", [128, 1], dt.bfloat16, isOutput=False)
    onesr_d = nc.declare_dram_parameter("onesr", [1, 128], dt.float32, isOutput=False)
    out_d = nc.declare_dram_parameter("out_t", [QW, S], dt.float32, isOutput=True)

    with tile.TileContext(nc) as tc:
        with (
            tc.tile_pool(name="const", bufs=1) as cpool,
            tc.tile_pool(name="qkv", bufs=1) as qkvpool,
            tc.tile_pool(name="att", bufs=1) as attpool,
            tc.tile_pool(name="dram", bufs=1, space="DRAM") as dpool,
        ):
            # ---- small resident constants ----
            cosd = cpool.tile([HD, S], dt.bfloat16)
            sind = cpool.tile([HD, S], dt.bfloat16)
            swapt = cpool.tile([HD, HD], dt.bfloat16)
            ident = cpool.tile([HD, HD], dt.bfloat16)
            dmask = cpool.tile([128, 128], dt.float32)
            onesc = cpool.tile([128, 1], dt.bfloat16)
            onesr = cpool.tile([1, 128], dt.float32)
            nc.sync.dma_start(cosd[:], cosd_d[:, :])
            nc.sync.dma_start(sind[:], sind_d[:, :])
            nc.sync.dma_start(swapt[:], swapt_d[:, :])
            nc.sync.dma_start(ident[:], ident_d[:, :])
            nc.sync.dma_start(dmask[:], dmask_d[:, :])
            nc.sync.dma_start(onesc[:], onesc_d[:, :])
            nc.sync.dma_start(onesr[:], onesr_d[:, :])
            # warm up the ACT exp table load before attention needs it
            warm = cpool.tile([1, 1], dt.float32)
            nc.scalar.activation(warm[:], onesr[0:1, 0:1],
                                 mybir.ActivationFunctionType.Exp)

            # ---- persistent activations ----
            qt = [qkvpool.tile([HD, S], dt.bfloat16, name=f"qt{h}", tag=f"qt{h}")
                  for h in range(HL)]
            kt = qkvpool.tile([HD, S], dt.bfloat16)
            vv = qkvpool.tile([128, NT, HD], dt.bfloat16)   # [t_part, ti, hd]
            att = [attpool.tile([HD, S], dt.bfloat16, name=f"att{h}", tag=f"att{h}")
                   for h in range(HL)]

            xt_r = xt_d[:, :].rearrange("(k p) s -> p k s", p=128)

            # ================= phase 1: projections + rope =================
            with (
                tc.tile_pool(name="w1", bufs=1) as wpool,
                tc.tile_pool(name="xc", bufs=2) as xpool,
                tc.tile_pool(name="p1", bufs=3, space="PSUM") as pp1,
                tc.tile_pool(name="pr", bufs=2, space="PSUM") as ppr,
                tc.tile_pool(name="rtmp", bufs=3) as rtpool,
            ):
                wqt = wpool.tile([128, KD, QW], dt.bfloat16)
                wkt = wpool.tile([128, KD, HD], dt.bfloat16)
                wvt = wpool.tile([128, KD, HD], dt.bfloat16)
                vt = wpool.tile([HD, S], dt.bfloat16)
                wqt_r = wqt_d[:, :].rearrange("(k p) n -> p k n", p=128)
                # interleave first x-chunk with weight loads so the first
                # matmuls can start as early as possible
                xc0 = xpool.tile([128, KD, SC], dt.bfloat16, tag="xc")
                for kg in range(4):
                    ksl = slice(kg * 8, (kg + 1) * 8)
                    nc.sync.dma_start(xc0[:, ksl, :], xt_r[:, ksl, 0:SC])
                    nc.sync.dma_start(wqt[:, ksl, :], wqt_r[:, ksl, :])
                nc.sync.dma_start(
                    wkt[:], wkt_d[:, :].rearrange("(k p) n -> p k n", p=128))
                nc.sync.dma_start(
                    wvt[:], wvt_d[:, :].rearrange("(k p) n -> p k n", p=128))

                for sc in range(NSC):
                    ssl = slice(sc * SC, (sc + 1) * SC)
                    if sc == 0:
                        xc = xc0
                    else:
                        xc = xpool.tile([128, KD, SC], dt.bfloat16, tag="xc")
                        for kg in range(4):
                            ksl = slice(kg * 8, (kg + 1) * 8)
                            nc.sync.dma_start(xc[:, ksl, :], xt_r[:, ksl, ssl])

                    # 4 Q heads (rope), K (rope), V (plain) — all [hd, s]
                    for hi in range(HL + 2):
                        ps = pp1.tile([128, SC], dt.float32)
                        for k in range(KD):
                            if hi < HL:
                                lhs = wqt[:, k, hi * HD:(hi + 1) * HD]
                            elif hi == HL:
                                lhs = wkt[:, k, :]
                            else:
                                lhs = wvt[:, k, :]
                            nc.tensor.matmul(ps[:], lhs, xc[:, k, :],
                                             start=(k == 0), stop=(k == KD - 1))
                        if hi == HL + 1:
                            nc.scalar.copy(vt[:, ssl], ps[:])
                            continue
                        # rope: out = q*cos + rot(q)*sin, rot via swap-matmul
                        qs = rtpool.tile([128, SC], dt.bfloat16, tag="ropeqs")
                        qc = rtpool.tile([128, SC], dt.bfloat16, tag="ropeqc")
                        nc.vector.tensor_mul(qs[:], ps[:], sind[:, ssl])
                        nc.vector.tensor_mul(qc[:], ps[:], cosd[:, ssl])
                        ps2 = ppr.tile([128, SC], dt.float32)
                        nc.tensor.matmul(ps2[:], swapt[:], qs[:], start=True, stop=False)
                        nc.tensor.matmul(ps2[:], ident[:], qc[:], start=False, stop=True)
                        dst = qt[hi] if hi < HL else kt
                        nc.scalar.copy(dst[:, ssl], ps2[:])

                    # V tiles in [t, hd] layout via DMA transpose
                    for vtile in range(4):
                        ti = sc * 4 + vtile
                        nc.sync.dma_start_transpose(
                            vv[:, ti, :], vt[:, ti * 128:(ti + 1) * 128])

            # ============ phase 2+3: attention, allgather, out-proj ============
            with (
                tc.tile_pool(name="wo", bufs=1) as wopool,
                tc.tile_pool(name="agc", bufs=2) as agpool,
                tc.tile_pool(name="st", bufs=2, space="PSUM") as stpool,
                tc.tile_pool(name="pv", bufs=2, space="PSUM") as pvpool,
                tc.tile_pool(name="rs", bufs=2, space="PSUM") as rspool,
                tc.tile_pool(name="p3", bufs=2, space="PSUM") as pp3,
                tc.tile_pool(name="pt", bufs=4) as ptpool,
                tc.tile_pool(name="ep", bufs=2) as eppool,
                tc.tile_pool(name="o3", bufs=3) as opool,
            ):
                wot = wopool.tile([128, KD, QW], dt.bfloat16)
                nc.sync.dma_start(
                    wot[:], wot_d[:, :].rearrange("(k p) n -> p k n", p=128))

                def epilogue_a(sc, h, pv, rs):
                    # reciprocal as soon as rowsums land (frees rs quickly)
                    rec = eppool.tile([1, SC], dt.float32, tag="rec")
                    nc.vector.reciprocal(rec[:], rs[:])
                    return (sc, h, pv, rec)

                def epilogue_b(sc, h, pv, rec):
                    # normalize columns of attnT by 1/rowsum; the rank-1
                    # broadcast rides PE (cheap) well after rec is ready
                    ssl = slice(sc * SC, (sc + 1) * SC)
                    bc = pp3.tile([128, SC], dt.float32, tag="ps3")
                    nc.tensor.matmul(bc[:], onesr[:], rec[:], start=True, stop=True)
                    bcs = eppool.tile([128, SC], dt.float32, tag="bcs")
                    nc.scalar.copy(bcs[:], bc[:])
                    nc.vector.tensor_mul(att[h][:, ssl], pv[:], bcs[:])

                def allgather_half(sc, half):
                    # gather this core's head pair (2*half, 2*half+1):
                    # out block r covers global i-tiles {4r+2*half, 4r+2*half+1}
                    ssl = slice(sc * SC, (sc + 1) * SC)
                    ag_in = dpool.tile([2 * HD, SC], dt.bfloat16,
                                       name=f"agi{sc}{half}", tag=f"agi{sc}{half}")
                    ag_out = dpool.tile([NCORES * 2 * HD, SC], dt.bfloat16,
                                        name=f"ago{sc}{half}", tag=f"ago{sc}{half}",
                                        addr_space="Shared")
                    for hh in range(2):
                        h = 2 * half + hh
                        nc.sync.dma_start(ag_in[hh * HD:(hh + 1) * HD, :],
                                          att[h][:, ssl])
                    nc.gpsimd.collective_compute(
                        "AllGather",
                        mybir.AluOpType.bypass,
                        replica_groups=[list(range(NCORES))],
                        ins=[ag_in.opt()],
                        outs=[ag_out.opt()],
                    )
                    return ag_out

                def outproj(sc, ag_a, ag_b):
                    ssl = slice(sc * SC, (sc + 1) * SC)
                    agcs = []
                    for ag in (ag_a, ag_b):
                        ag_r = ag[:, :].rearrange("(m p) s -> p m s", p=128)
                        agc = agpool.tile([128, NT, SC], dt.bfloat16, tag="agc")
                        nc.sync.dma_start(agc[:], ag_r[:, :, :])
                        agcs.append(agc)
                    for oc in range(4):
                        ps = pp3.tile([128, SC], dt.float32, tag="ps3")
                        for half in range(2):
                            for m in range(NT):
                                kg = (m // 2) * 4 + (m % 2) + 2 * half
                                nc.tensor.matmul(
                                    ps[:], wot[:, kg, oc * 128:(oc + 1) * 128],
                                    agcs[half][:, m, :],
                                    start=(half == 0 and m == 0),
                                    stop=(half == 1 and m == NT - 1))
                        ot = opool.tile([128, SC], dt.float32)
                        nc.vector.tensor_copy(ot[:], ps[:])
                        nc.sync.dma_start(out_d[oc * 128:(oc + 1) * 128, ssl], ot[:])

                pending_op = None   # (sc, ag_a, ag_b) awaiting outproj
                pending_ep = None   # deferred epilogue_b
                ag_a_cur = None
                for sc in range(NSC):
                    n_t = sc * 4 + 4
                    for h in range(HL):
                        pv = pvpool.tile([128, SC], dt.float32)
                        rs = rspool.tile([1, SC], dt.float32)
                        for ti in range(n_t):
                            # valid columns: s >= ti*128 (ti==0 covers all)
                            d_off = ti * 128 - sc * SC
                            v0 = max(d_off, 0)
                            vsl = slice(v0, SC)
                            qcl = slice(sc * SC + v0, (sc + 1) * SC)
                            st = stpool.tile([128, SC], dt.float32)
                            nc.tensor.matmul(st[:, vsl],
                                             kt[:, ti * 128:(ti + 1) * 128],
                                             qt[h][:, qcl], start=True, stop=True)
                            if d_off >= 0:
                                nc.vector.tensor_add(st[:, d_off:d_off + 128],
                                                     st[:, d_off:d_off + 128],
                                                     dmask[:])
                            pt = ptpool.tile([128, SC], dt.bfloat16)
                            nc.scalar.activation(pt[:, vsl], st[:, vsl],
                                                 mybir.ActivationFunctionType.Exp,
                                                 scale=SCALE)
                            nc.tensor.matmul(rs[:, vsl], onesc[:], pt[:, vsl],
                                             start=(ti == 0), stop=(ti == n_t - 1))
                            nc.tensor.matmul(pv[:, vsl], vv[:, ti, :], pt[:, vsl],
                                             start=(ti == 0), stop=(ti == n_t - 1))
                        ep = epilogue_a(sc, h, pv, rs)
                        if pending_ep is not None:
                            epilogue_b(*pending_ep)
                            pending_ep = None
                        pending_ep = ep
                        if h == 2:
                            # epilogue_b(h0) and (h1) are done by now
                            ag_a_cur = allgather_half(sc, 0)
                        if h == 3 and pending_op is not None:
                            outproj(*pending_op)
                            pending_op = None
                    epilogue_b(*pending_ep)
                    pending_ep = None
                    ag_b_cur = allgather_half(sc, 1)
                    pending_op = (sc, ag_a_cur, ag_b_cur)
                outproj(*pending_op)
    if not nc.is_finalized():
        nc.finalize()
    return nc


_CACHE = {}


def _get_nc():
    if "nc" not in _CACHE:
        _CACHE["nc"] = _build_nc()
    return _CACHE["nc"]


def _prep_in_maps(x, wq, wk, wv, wo, freqs_cos, freqs_sin):
    xt = np.ascontiguousarray(x.reshape(S, D).T).astype(BF)
    cosd = np.repeat(np.asarray(freqs_cos, np.float32).T, 2, axis=0).astype(BF)
    sind = np.repeat(np.asarray(freqs_sin, np.float32).T, 2, axis=0).astype(BF)
    swapt = np.zeros((HD, HD), np.float32)
    for i in range(HD // 2):
        swapt[2 * i + 1, 2 * i] = -1.0
        swapt[2 * i, 2 * i + 1] = 1.0
    swapt = swapt.astype(BF)
    ident = np.eye(HD, dtype=np.float32).astype(BF)
    t_idx = np.arange(128)[:, None]
    s_idx = np.arange(128)[None, :]
    dmask = np.where(s_idx >= t_idx, 0.0, NEG).astype(np.float32)
    onesc = np.ones((128, 1), np.float32).astype(BF)
    onesr = np.ones((1, 128), np.float32)

    wq = np.asarray(wq, np.float32)
    wk = np.asarray(wk, np.float32)
    wv = np.asarray(wv, np.float32)
    wo = np.asarray(wo, np.float32)

    in_maps = []
    for c in range(NCORES):
        qsl = slice(QW * c, QW * (c + 1))
        ksl = slice(HD * c, HD * (c + 1))
        in_maps.append({
            "xt": xt,
            "wqt": np.ascontiguousarray(wq[qsl].T).astype(BF),
            "wkt": np.ascontiguousarray(wk[ksl].T).astype(BF),
            "wvt": np.ascontiguousarray(wv[ksl].T).astype(BF),
            "wot": np.ascontiguousarray(wo[qsl].T).astype(BF),
            "cosd": cosd, "sind": sind, "swapt": swapt, "ident": ident,
            "dmask": dmask, "onesc": onesc, "onesr": onesr,
        })
    return in_maps


def run(inputs, trace=False):
    from concourse.bass_utils import run_bass_kernel_spmd
    nc = _get_nc()
    in_maps = _prep_in_maps(
        inputs["x"], inputs["wq"], inputs["wk"], inputs["wv"], inputs["wo"],
        inputs["freqs_cos"], inputs["freqs_sin"])
    res = run_bass_kernel_spmd(nc, in_maps, core_ids=list(range(NCORES)),
                               trace=trace)
    shards = [np.asarray(res.results[c]["out_t"], np.float32)
              for c in range(NCORES)]
    full = np.concatenate(shards, axis=0)          # [4096, 2048]
    out = np.ascontiguousarray(full.T)[None]       # [1, 2048, 4096]
    return out.astype(np.float32), res


def kernel(**inputs):
    out, _ = run(inputs, trace=False)
    return out
